# revision 1
# baseline (speedup 1.0000x reference)
"""Local (windowed) attention Trainium2 Bass kernel.

Problem: q,k,v [8, 8, 4096, 64] fp32; window 128, look_backward 1, pad -1.0.
out[b,h,w,i,:] = softmax(scale * q_wi . [k_{w-1}; k_w]) @ [v_{w-1}; v_w]
(with window -1 = all -1.0 pad values, which DO enter the softmax).

Sharding: data-parallel over flat batch*heads (64) -> 8 heads per core.

Per-core layouts (prepared host-side):
  qT : [4, 128, 4096]  float16 - head pair stacked on partitions (d=64 each),
                                 free axis = 4096 queries (d-major transposed)
  kT : [4, 128, 4224]  float16 - same, with one pad chunk (128 keys of -1.0)
                                 prepended -> 33 chunks of 128 keys
  v  : [8, 128, 33, 65] float16 - per head; partition = key-within-chunk,
                                 pad chunk prepended; col 64 = 1.0 (ones
                                 column yields softmax denominator l)
  out: [8, 128, 32, 64] float32 - partition = query-within-window

Device pipeline per head pair, per key chunk p (0..32):
  MM1 (fp16): scoresT[j, i] for the <=2 windows attending chunk p
              lhsT = kT chunk [64,128], rhs = qT slice [64,<=256];
              heads of a pair alternate PE row groups (base partition 0/64);
              each PSUM bank only ever sees one weight base partition
              (mixing row-group bases within a bank hard-crashes the device).
  ACT exp (scale=1/8) psum -> fp16 P tiles (batched 2 chunks x 2 heads)
  MM2 (fp16): out_w[i, 0:65] += P_blockT @ v_aug[p]  (col 64 accumulates l)
  DVE: evacuate completed windows psum -> sbuf; per 8 windows: reciprocal(l),
       broadcast-multiply, contiguous DMA store.

Accuracy: ~5e-4 relative (fp16 operand rounding; the 1/8 softmax scale keeps
logit perturbation ~4e-4, exact fp32 PSUM accumulation everywhere).
"""

import os
import sys

for _p in ("/opt/trn_rl_repo", "/opt/pypackages"):
    if os.path.isdir(_p) and _p not in sys.path:
        sys.path.append(_p)

import numpy as np

import concourse.mybir as mybir
import concourse.tile as tile
from concourse import bacc
import concourse.bass_utils as _bu
from concourse.bass_utils import run_bass_kernel_spmd

# Enable walrus LDWEIGHTS optimization (background weight-buffer loads) so
# matmul weight loads overlap streaming; the repo default disables it.
if os.environ.get("KERNEL_LDW_OPT", "0") == "1":
    _orig_run_command = _bu.run_command

    def _run_command_ldw(cmd, **kw):
        if isinstance(cmd, list):
            cmd = ["--enable-ldw-opt=true" if c == "--enable-ldw-opt=false"
                   else c for c in cmd]
        return _orig_run_command(cmd, **kw)

    _bu.run_command = _run_command_ldw

B, H, N, D = 8, 8, 4096, 64
WS = 128                 # window size
W = N // WS              # 32 windows
C = W + 1                # 33 key chunks incl. pad chunk
NC = 8                   # cores
HPC = (B * H) // NC      # 8 heads per core
PAIRS = HPC // 2         # 4 head pairs per core
SCALE = float(D) ** -0.5

MM1_DT = mybir.dt.float16
MM2_DT = mybir.dt.float16
GROUP = 2                # key chunks per exp batch

_NC_CACHE = {}


def build_nc(pairs=PAIRS, w=W):
    c = w + 1
    n = w * WS
    nc = bacc.Bacc("TRN2", target_bir_lowering=False)
    qT = nc.dram_tensor("qT", [pairs, 128, n], MM1_DT, kind="ExternalInput")
    kT = nc.dram_tensor("kT", [pairs, 128, c * WS], MM1_DT, kind="ExternalInput")
    vv = nc.dram_tensor("v", [2 * pairs, 128, c, D + 1], MM2_DT, kind="ExternalInput")
    out = nc.dram_tensor("out", [2 * pairs, 128, w, D], mybir.dt.float32,
                         kind="ExternalOutput")

    f32 = mybir.dt.float32
    Exp = mybir.ActivationFunctionType.Exp

    with tile.TileContext(nc) as tc:
        with (
            tc.tile_pool(name="qk", bufs=2) as qk_pool,
            tc.tile_pool(name="vp", bufs=4) as v_pool,
            tc.tile_pool(name="pt", bufs=3) as pt_pool,
            tc.tile_pool(name="un", bufs=4) as un_pool,
            tc.tile_pool(name="st", bufs=2) as st_pool,
            tc.tile_pool(name="rc", bufs=2) as rc_pool,
            tc.tile_pool(name="ps_s", bufs=2, space="PSUM") as ps_s,
            tc.tile_pool(name="ps_o", bufs=4, space="PSUM") as ps_o,
        ):
            for pair in range(pairs):
                qt = qk_pool.tile([128, n], MM1_DT, tag="qT")
                kt = qk_pool.tile([128, c * WS], MM1_DT, tag="kT")
                NSL = 8 if pair == 0 else 4
                ck, cq = c * WS // NSL, n // NSL

                def load_slice(sl):
                    nc.sync.dma_start(kt[:, sl * ck:(sl + 1) * ck],
                                      kT[pair][:, sl * ck:(sl + 1) * ck])
                    nc.sync.dma_start(qt[:, sl * cq:(sl + 1) * cq],
                                      qT[pair][:, sl * cq:(sl + 1) * cq])

                load_slice(0)
                # interleave v halves between input slices: HWDGE DMAs drain
                # FIFO per engine, so a monolithic v load would delay the
                # kt/qt slices that feed the next MM1s
                vts = [v_pool.tile([128, c, D + 1], MM2_DT, tag="v",
                                   name=f"v_{pair}_{h}") for h in range(2)]
                ch = c // 2
                for h in range(2):
                    nc.sync.dma_start(vts[h][:, 0:ch], vv[2 * pair + h][:, 0:ch])
                load_slice(1)
                for h in range(2):
                    nc.sync.dma_start(vts[h][:, ch:], vv[2 * pair + h][:, ch:])
                for sl in range(2, NSL):
                    load_slice(sl)

                unnorm = [un_pool.tile([128, w, D + 1], f32, tag="un",
                                       name=f"un_{pair}_{h}")
                          for h in range(2)]
                out_ps = {}  # (h, w) -> psum tile

                EB = 8

                def emit_epilogue(h, w0, nb):
                    recip = rc_pool.tile([128, EB], f32, tag="recip",
                                         name=f"rc_{pair}_{h}_{w0}")
                    nc.vector.reciprocal(recip[:, 0:nb],
                                         unnorm[h][:, w0:w0 + nb, D])
                    stg = st_pool.tile([128, EB, D], f32, tag="stg",
                                       name=f"st_{pair}_{h}_{w0}")
                    nc.vector.tensor_mul(
                        stg[:, 0:nb],
                        unnorm[h][:, w0:w0 + nb, 0:D],
                        recip[:, 0:nb, None].to_broadcast((128, nb, D)),
                    )
                    nc.sync.dma_start(out[2 * pair + h][:, w0:w0 + nb],
                                      stg[:, 0:nb])

                groups = [list(range(g, min(g + GROUP, c)))
                          for g in range(0, c, GROUP)]
                pending_mm2 = None

                def emit_mm2s(chunks, pt):
                    do_mm2s(chunks, pt)

                for chunks in groups:
                    ps = ps_s.tile([128, GROUP * 2 * 256], f32, tag="scores")
                    # MM1s
                    runs = []  # written (col, n) regions
                    for s, p in enumerate(chunks):
                        qlo = max(0, (p - 1) * WS)
                        qhi = min(n, (p + 1) * WS)
                        if p == 0:
                            qhi = min(n, 2 * WS)  # avoid garbage: fill 256
                        nq = qhi - qlo
                        for h in range(2):
                            col = h * (GROUP * 256) + s * 256
                            nc.tensor.matmul(
                                ps[:, col:col + nq],
                                kt[64 * h:64 * h + 64, p * WS:(p + 1) * WS],
                                qt[64 * h:64 * h + 64, qlo:qhi],
                                start=True, stop=True,
                            )
                            runs.append((col, nq))
                    # batched exp: merge adjacent written runs
                    pt = pt_pool.tile([128, GROUP * 2 * 256], MM2_DT, tag="pt")
                    merged = []
                    for rcol, rn in sorted(runs):
                        if merged and merged[-1][0] + merged[-1][1] == rcol:
                            merged[-1][1] += rn
                        else:
                            merged.append([rcol, rn])
                    for rcol, rn in merged:
                        nc.scalar.activation(pt[:, rcol:rcol + rn],
                                             ps[:, rcol:rcol + rn],
                                             Exp, scale=SCALE)
                    # MM2s + evacuation (deferred one group for pipelining)
                    def do_mm2s(chunks, pt):
                      for s, p in enumerate(chunks):
                        for h in range(2):
                            col = h * (GROUP * 256) + s * 256
                            if p >= 1:
                                # block 0: window p-1 self-contribution (last)
                                wi = p - 1
                                t = out_ps[(h, wi)]
                                nc.tensor.matmul(
                                    t[:, 0:D + 1],
                                    pt[:, col:col + WS],
                                    vts[h][:, p, :],
                                    start=False, stop=True,
                                )
                                nc.vector.tensor_copy(unnorm[h][:, wi, :],
                                                      t[:, 0:D + 1])
                                del out_ps[(h, wi)]
                                eb = 8
                                if (wi + 1) % eb == 0 or wi == w - 1:
                                    emit_epilogue(h, wi - wi % eb,
                                                  wi % eb + 1)
                            if p <= w - 1:
                                # window p prev-contribution (first)
                                bcol = col + (WS if p >= 1 else 0)
                                t = ps_o.tile([128, D + 1], f32, tag="out")
                                out_ps[(h, p)] = t
                                nc.tensor.matmul(
                                    t[:, 0:D + 1],
                                    pt[:, bcol:bcol + WS],
                                    vts[h][:, p, :],
                                    start=True, stop=False,
                                )
                    if pending_mm2 is not None:
                        emit_mm2s(*pending_mm2)
                    pending_mm2 = (chunks, pt)
                if pending_mm2 is not None:
                    emit_mm2s(*pending_mm2)
                    pending_mm2 = None

    nc.compile()
    return nc


def _get_nc():
    if "nc" not in _NC_CACHE:
        _NC_CACHE["nc"] = build_nc()
    return _NC_CACHE["nc"]


def _prep_core(qf, kf, vf, lo):
    """Build one core's input dict from flat [64, 4096, 64] fp32 arrays."""
    q8 = qf[lo:lo + HPC]                      # [8, 4096, 64]
    k8 = kf[lo:lo + HPC]
    v8 = vf[lo:lo + HPC]

    qT = np.ascontiguousarray(q8.transpose(0, 2, 1)).reshape(PAIRS, 128, N)
    qT = qT.astype(np.float16)

    pad = np.full((HPC, WS, D), -1.0, dtype=np.float32)
    kp = np.concatenate([pad, k8], axis=1)    # [8, 4224, 64]
    kT = np.ascontiguousarray(kp.transpose(0, 2, 1)).reshape(PAIRS, 128, C * WS)
    kT = kT.astype(np.float16)

    vp = np.concatenate([pad, v8], axis=1)    # [8, 4224, 64]
    ones = np.ones((HPC, C * WS, 1), dtype=np.float32)
    va = np.concatenate([vp, ones], axis=2)   # [8, 4224, 65]
    va = va.reshape(HPC, C, WS, D + 1).transpose(0, 2, 1, 3)  # [8, 128, 33, 65]
    va = np.ascontiguousarray(va).astype(np.float16)

    return {"qT": qT, "kT": kT, "v": va}


def kernel(q, k, v):
    q = np.asarray(q, dtype=np.float32)
    k = np.asarray(k, dtype=np.float32)
    v = np.asarray(v, dtype=np.float32)
    qf = q.reshape(B * H, N, D)
    kf = k.reshape(B * H, N, D)
    vf = v.reshape(B * H, N, D)

    nc = _get_nc()
    in_maps = [_prep_core(qf, kf, vf, HPC * c) for c in range(NC)]
    res = run_bass_kernel_spmd(nc, in_maps, core_ids=list(range(NC)))

    outs = []
    for c in range(NC):
        o = res.results[c]["out"]             # [8, 128, 32, 64]
        o = o.transpose(0, 2, 1, 3).reshape(HPC, N, D)
        outs.append(o)
    return np.concatenate(outs, axis=0).reshape(B, H, N, D).astype(np.float32)


if __name__ == "__main__":
    rng = np.random.default_rng(0)
    q = rng.standard_normal((B, H, N, D), dtype=np.float32)
    k = rng.standard_normal((B, H, N, D), dtype=np.float32)
    v = rng.standard_normal((B, H, N, D), dtype=np.float32)
    o = kernel(q, k, v)
    print("out", o.shape, o.dtype, float(np.abs(o).max()))



# revision 2
# speedup vs baseline: 1.2437x; 1.2437x over previous
"""Local (windowed) attention Trainium2 Bass kernel.

Problem: q,k,v [8, 8, 4096, 64] fp32; window 128, look_backward 1, pad -1.0.
out[b,h,w,i,:] = softmax(scale * q_wi . [k_{w-1}; k_w]) @ [v_{w-1}; v_w]
(with window -1 = all -1.0 pad values, which DO enter the softmax).

Sharding: data-parallel over flat batch*heads (64) -> 8 heads per core.

Per-core layouts (prepared host-side):
  qT : [4, 128, 4096]  float16 - head pair stacked on partitions (d=64 each),
                                 free axis = 4096 queries (d-major transposed)
  kT : [4, 128, 4224]  float16 - same, with one pad chunk (128 keys of -1.0)
                                 prepended -> 33 chunks of 128 keys
  v  : [8, 128, 33, 65] float16 - per head; partition = key-within-chunk,
                                 pad chunk prepended; col 64 = 1.0 (ones
                                 column yields softmax denominator l)
  out: [8, 128, 32, 64] float16 - partition = query-within-window (host
                                 upcasts to fp32)

Device pipeline per head pair, per key chunk p (0..32):
  MM1 (fp16): scoresT[j, i] for the <=2 windows attending chunk p
              lhsT = kT chunk [64,128], rhs = qT slice [64,<=256];
              heads of a pair alternate PE row groups (base partition 0/64);
              each PSUM bank only ever sees one weight base partition
              (mixing row-group bases within a bank hard-crashes the device).
  ACT exp (scale=1/8) one full-tile activation per 2-chunk group
              psum -> fp16 P tiles (garbage cols exp'd too; never consumed)
  MM2 (fp16): out_w[i, 0:65] += P_blockT @ v_aug[p]  (col 64 accumulates l)
              into per-(head, 7-window-batch) psum accumulators [128, 7, 65]
              (one 2KB bank each)
  DVE: per 7-window batch: reciprocal(l) + broadcast-multiply psum -> fp16
       staging sbuf; one contiguous DMA store per head at end of pair.

Accuracy: ~6e-4 relative (fp16 operand rounding + fp16 output; the 1/8
softmax scale keeps logit perturbation ~4e-4, fp32 PSUM accumulation).
"""

import os
import sys

for _p in ("/opt/trn_rl_repo", "/opt/pypackages"):
    if os.path.isdir(_p) and _p not in sys.path:
        sys.path.append(_p)

import numpy as np

import concourse.mybir as mybir
import concourse.tile as tile
from concourse import bacc
from concourse.bass_utils import run_bass_kernel_spmd

B, H, N, D = 8, 8, 4096, 64
WS = 128                 # window size
W = N // WS              # 32 windows
C = W + 1                # 33 key chunks incl. pad chunk
NC = 8                   # cores
HPC = (B * H) // NC      # 8 heads per core
PAIRS = HPC // 2         # 4 head pairs per core
SCALE = float(D) ** -0.5

MM1_DT = mybir.dt.float16
MM2_DT = mybir.dt.float16
GROUP = 2                # key chunks per exp batch
EB = 7                   # windows per psum out-accumulator bank (7*65*4B<=2KB)

_NC_CACHE = {}


def build_nc(pairs=PAIRS, w=W):
    c = w + 1
    n = w * WS
    nb_batches = (w + EB - 1) // EB
    nc = bacc.Bacc("TRN2", target_bir_lowering=False)
    qT = nc.dram_tensor("qT", [pairs, 128, n], MM1_DT, kind="ExternalInput")
    kT = nc.dram_tensor("kT", [pairs, 128, c * WS], MM1_DT, kind="ExternalInput")
    vv = nc.dram_tensor("v", [2 * pairs, 128, c, D + 1], MM2_DT, kind="ExternalInput")
    out = nc.dram_tensor("out", [2 * pairs, 128, w, D], mybir.dt.float16,
                         kind="ExternalOutput")

    f32 = mybir.dt.float32
    Exp = mybir.ActivationFunctionType.Exp

    with tile.TileContext(nc) as tc:
        with (
            tc.tile_pool(name="qk", bufs=2) as qk_pool,
            tc.tile_pool(name="vp", bufs=4) as v_pool,
            tc.tile_pool(name="pt", bufs=3) as pt_pool,
            tc.tile_pool(name="st", bufs=4) as st_pool,
            tc.tile_pool(name="rc", bufs=4) as rc_pool,
            tc.tile_pool(name="ps_s", bufs=2, space="PSUM") as ps_s,
            tc.tile_pool(name="ps_o", bufs=4, space="PSUM") as ps_o,
        ):
            for pair in range(pairs):
                qt = qk_pool.tile([128, n], MM1_DT, tag="qT")
                kt = qk_pool.tile([128, c * WS], MM1_DT, tag="kT")
                NSL = 8 if pair == 0 else 2
                ck, cq = c * WS // NSL, n // NSL

                def load_slice(sl):
                    nc.sync.dma_start(kt[:, sl * ck:(sl + 1) * ck],
                                      kT[pair][:, sl * ck:(sl + 1) * ck])
                    nc.sync.dma_start(qt[:, sl * cq:(sl + 1) * cq],
                                      qT[pair][:, sl * cq:(sl + 1) * cq])

                load_slice(0)
                # interleave v halves between input slices: HWDGE DMAs drain
                # FIFO per engine, so a monolithic v load would delay the
                # kt/qt slices that feed the next MM1s
                vts = [v_pool.tile([128, c, D + 1], MM2_DT, tag="v",
                                   name=f"v_{pair}_{h}") for h in range(2)]
                ch = c // 2
                for h in range(2):
                    nc.sync.dma_start(vts[h][:, 0:ch], vv[2 * pair + h][:, 0:ch])
                if NSL > 1:
                    load_slice(1)
                for h in range(2):
                    nc.sync.dma_start(vts[h][:, ch:], vv[2 * pair + h][:, ch:])
                for sl in range(2, NSL):
                    load_slice(sl)

                stg = [st_pool.tile([128, w, D], MM2_DT, tag="stg",
                                    name=f"stg_{pair}_{h}") for h in range(2)]
                accum = {}  # (h, batch) -> psum accumulation tile

                def emit_evac(h, b):
                    nb = min(EB, w - b * EB)
                    acc = accum.pop((h, b))
                    rc = rc_pool.tile([128, EB], f32, tag="rc",
                                      name=f"rc_{pair}_{h}_{b}")
                    nc.vector.reciprocal(rc[:, 0:nb], acc[:, 0:nb, D])
                    nc.vector.tensor_mul(
                        stg[h][:, b * EB:b * EB + nb],
                        acc[:, 0:nb, 0:D],
                        rc[:, 0:nb, None].to_broadcast((128, nb, D)),
                    )

                groups = [list(range(g, min(g + GROUP, c)))
                          for g in range(0, c, GROUP)]
                pending_mm2 = None

                def do_mm2s(chunks, pt):
                    for s, p in enumerate(chunks):
                        for h in range(2):
                            col = h * (GROUP * 256) + s * 256
                            if p >= 1:
                                # window p-1 self-contribution (stop)
                                wi = p - 1
                                t = accum[(h, wi // EB)]
                                nc.tensor.matmul(
                                    t[:, wi % EB, :],
                                    pt[:, col:col + WS],
                                    vts[h][:, p, :],
                                    start=False, stop=True,
                                )
                                if wi % EB == EB - 1 or wi == w - 1:
                                    emit_evac(h, wi // EB)
                            if p <= w - 1:
                                # window p prev-contribution (start)
                                bcol = col + (WS if p >= 1 else 0)
                                t = accum.get((h, p // EB))
                                if t is None:
                                    t = ps_o.tile([128, EB, D + 1], f32,
                                                  tag="out",
                                                  name=f"acc_{pair}_{h}_{p // EB}")
                                    accum[(h, p // EB)] = t
                                nc.tensor.matmul(
                                    t[:, p % EB, :],
                                    pt[:, bcol:bcol + WS],
                                    vts[h][:, p, :],
                                    start=True, stop=False,
                                )

                for chunks in groups:
                    ps = ps_s.tile([128, GROUP * 2 * 256], f32, tag="scores")
                    # MM1s
                    for s, p in enumerate(chunks):
                        qlo = max(0, (p - 1) * WS)
                        qhi = min(n, (p + 1) * WS)
                        if p == 0:
                            qhi = min(n, 2 * WS)  # avoid garbage: fill 256
                        nq = qhi - qlo
                        for h in range(2):
                            col = h * (GROUP * 256) + s * 256
                            nc.tensor.matmul(
                                ps[:, col:col + nq],
                                kt[64 * h:64 * h + 64, p * WS:(p + 1) * WS],
                                qt[64 * h:64 * h + 64, qlo:qhi],
                                start=True, stop=True,
                            )
                    # one full-tile exp; garbage cols (last chunk's upper
                    # half) are exp'd but never consumed by MM2
                    pt = pt_pool.tile([128, GROUP * 2 * 256], MM2_DT, tag="pt")
                    nc.scalar.activation(pt, ps, Exp, scale=SCALE)
                    # MM2s deferred one group for pipelining
                    if pending_mm2 is not None:
                        do_mm2s(*pending_mm2)
                    pending_mm2 = (chunks, pt)
                if pending_mm2 is not None:
                    do_mm2s(*pending_mm2)
                    pending_mm2 = None

                for h in range(2):
                    nc.sync.dma_start(out[2 * pair + h], stg[h])

    nc.compile()
    return nc


def _get_nc():
    if "nc" not in _NC_CACHE:
        _NC_CACHE["nc"] = build_nc()
    return _NC_CACHE["nc"]


def _prep_core(qf, kf, vf, lo):
    """Build one core's input dict from flat [64, 4096, 64] fp32 arrays."""
    q8 = qf[lo:lo + HPC]                      # [8, 4096, 64]
    k8 = kf[lo:lo + HPC]
    v8 = vf[lo:lo + HPC]

    qT = np.ascontiguousarray(q8.transpose(0, 2, 1)).reshape(PAIRS, 128, N)
    qT = qT.astype(np.float16)

    pad = np.full((HPC, WS, D), -1.0, dtype=np.float32)
    kp = np.concatenate([pad, k8], axis=1)    # [8, 4224, 64]
    kT = np.ascontiguousarray(kp.transpose(0, 2, 1)).reshape(PAIRS, 128, C * WS)
    kT = kT.astype(np.float16)

    vp = np.concatenate([pad, v8], axis=1)    # [8, 4224, 64]
    ones = np.ones((HPC, C * WS, 1), dtype=np.float32)
    va = np.concatenate([vp, ones], axis=2)   # [8, 4224, 65]
    va = va.reshape(HPC, C, WS, D + 1).transpose(0, 2, 1, 3)  # [8, 128, 33, 65]
    va = np.ascontiguousarray(va).astype(np.float16)

    return {"qT": qT, "kT": kT, "v": va}


def kernel(q, k, v):
    q = np.asarray(q, dtype=np.float32)
    k = np.asarray(k, dtype=np.float32)
    v = np.asarray(v, dtype=np.float32)
    qf = q.reshape(B * H, N, D)
    kf = k.reshape(B * H, N, D)
    vf = v.reshape(B * H, N, D)

    nc = _get_nc()
    in_maps = [_prep_core(qf, kf, vf, HPC * c) for c in range(NC)]
    res = run_bass_kernel_spmd(nc, in_maps, core_ids=list(range(NC)))

    outs = []
    for c in range(NC):
        o = res.results[c]["out"].astype(np.float32)  # [8, 128, 32, 64]
        o = o.transpose(0, 2, 1, 3).reshape(HPC, N, D)
        outs.append(o)
    return np.concatenate(outs, axis=0).reshape(B, H, N, D).astype(np.float32)


if __name__ == "__main__":
    rng = np.random.default_rng(0)
    q = rng.standard_normal((B, H, N, D), dtype=np.float32)
    k = rng.standard_normal((B, H, N, D), dtype=np.float32)
    v = rng.standard_normal((B, H, N, D), dtype=np.float32)
    o = kernel(q, k, v)
    print("out", o.shape, o.dtype, float(np.abs(o).max()))
